# revision 1
# baseline (speedup 1.0000x reference)
"""Trainium2 Bass kernel for 2-layer GAT node classification (50K nodes, 800K edges).

Strategy:
  - Host: relabel nodes by in-degree into 392 tiles of 128; tile 8r+c -> core c
    round r, so all 8 cores share an identical program structure. Edges are
    grouped per destination; each destination's in-edges are split across two
    int16-indexable gather windows of the node table and laid out as per-tile
    grids (slot lane == destination lane).
  - Device (per layer): dense phase computes T[n] = [h fp16 | al_src f32 |
    al_dst f32] (512B rows), AllGather replicates T; edge phase dma_gathers
    source rows in 4096-row batches, computes w = exp(leaky_relu(als+ald))
    (no max subtraction; magnitudes are O(1)), scales messages, accumulates
    per-destination via identity matmuls in PSUM, flushes to SBUF
    accumulators; then normalizes by the summed weights, adds bias,
    activation.  Layer 2 feeds a classifier matmul + log_softmax.
  - Outputs are written contiguously (relabeled ids); host un-permutes.
"""
import sys

sys.path.insert(0, "/opt/trn_rl_repo")

import numpy as np

import concourse.bacc as bacc
import concourse.tile as tile
import concourse.mybir as mybir
from concourse.bass_utils import run_bass_kernel_spmd

P = 128
NCORES = 8
F_IN = 128
H = 4
C = 32
HC = 128
NCLS = 40
NEG = 0.2
EPS = 1e-16
J_MAX = 8  # gather chunks per dma_gather call (SWDGE desc ring limit: 1024 rows)

f32 = mybir.dt.float32
f16 = mybir.dt.float16
u16 = mybir.dt.uint16
i16 = mybir.dt.int16

LAST_EXEC_NS = None


# ---------------------------------------------------------------------------
# host preprocessing
# ---------------------------------------------------------------------------

def _cumcount(keys):
    """rank of each element among equal keys, input sorted by keys."""
    n = len(keys)
    if n == 0:
        return np.zeros(0, dtype=np.int64)
    first = np.ones(n, dtype=bool)
    first[1:] = keys[1:] != keys[:-1]
    idx = np.arange(n)
    start = np.maximum.accumulate(np.where(first, idx, 0))
    return idx - start


def _preprocess(x, edge_index, n_real):
    """Relabel nodes, build gather grids. Returns a struct dict."""
    n_tiles = -(-(n_real + 1) // P)  # at least one pad (the lo dummy)
    n_tiles = -(-n_tiles // NCORES) * NCORES  # divisible by NCORES
    npad = n_tiles * P
    tpc = n_tiles // NCORES
    npc = tpc * P
    table_rows = npad + P  # one extra tile of rows; row npad = hi dummy
    win = 32768
    if npad <= win:
        hi_base = 0  # single window world (small test sizes)
    else:
        hi_base = table_rows - win
        assert hi_base + win >= table_rows and npad - 1 - hi_base <= 32767
    lo_top = min(win, table_rows)  # lo window covers rows [0, lo_top)

    src0 = np.asarray(edge_index[0]).astype(np.int64)
    dst0 = np.asarray(edge_index[1]).astype(np.int64)

    deg = np.bincount(dst0, minlength=npad).astype(np.int64)
    deg[:n_real] += 1  # self loops

    order = np.argsort(deg, kind="stable")  # ascending; pads (deg 0) first
    pos = np.empty(npad, dtype=np.int64)
    pos[order] = np.arange(npad)
    tile_of = pos // P
    lane_of = pos % P
    r_of = tile_of // NCORES
    c_of = tile_of % NCORES
    new_id = c_of * npc + r_of * P + lane_of  # old -> new

    assert deg[np.flatnonzero(new_id == 0)[0]] == 0, "id 0 must be a pad"

    # relabeled edge list incl self loops
    all_src = np.concatenate([new_id[src0], new_id[:n_real]])
    all_dst = np.concatenate([new_id[dst0], new_id[:n_real]])

    # sort by destination
    o = np.argsort(all_dst, kind="stable")
    s = all_src[o]
    d = all_dst[o]

    # window classification (in new-id space)
    if hi_base == 0:
        cat = np.zeros(len(s), dtype=np.int8)  # everything lo
    else:
        cat = np.full(len(s), 2, dtype=np.int8)  # flex
        cat[s < hi_base] = 0
        cat[s >= lo_top] = 1

    ndeg = np.bincount(d, minlength=npad)
    nlo = np.bincount(d[cat == 0], minlength=npad)
    nhi = np.bincount(d[cat == 1], minlength=npad)
    # balanced per-destination split
    kl_node = np.maximum(nlo, np.minimum(ndeg - nhi, (ndeg + 1) // 2))
    kh_node = ndeg - kl_node

    # flex edges: rank among flex of same dst; first (kl_node - nlo) -> lo
    flex_rank = np.zeros(len(s), dtype=np.int64)
    mflex = cat == 2
    flex_rank[mflex] = _cumcount(d[mflex])
    to_lo = (cat == 0) | (mflex & (flex_rank < (kl_node - nlo)[d]))

    # slot rank within (dst, window)
    k_slot = np.zeros(len(s), dtype=np.int64)
    for m in (to_lo, ~to_lo):
        k_slot[m] = _cumcount(d[m])

    # per-round common K values
    def round_k(k_node):
        k_tile = np.max(k_node.reshape(n_tiles, P), axis=1)  # by new-id tile
        # new-id tile index t = c*tpc + r; round r common = max over c
        return np.max(k_tile.reshape(NCORES, tpc), axis=0)  # [tpc]

    KL = round_k(kl_node)
    KH = round_k(kh_node)
    if hi_base == 0:
        KH = np.zeros_like(KH)

    cumKL = np.concatenate([[0], np.cumsum(KL)])
    cumKH = np.concatenate([[0], np.cumsum(KH)])
    CL, CH = int(cumKL[-1]), int(cumKH[-1])

    DUMMY_LO = 0
    DUMMY_HI = npad - hi_base  # table row npad

    # slot streams per core: [n_chunks*128] int16 indices
    slots_lo = np.full((NCORES, CL * P), DUMMY_LO, dtype=np.int64)
    slots_hi = np.full((NCORES, CH * P), DUMMY_HI, dtype=np.int64)

    core_e = d // npc
    r_e = (d % npc) // P
    lane_e = d % P
    pos_lo = (cumKL[r_e] + k_slot) * P + lane_e
    pos_hi = (cumKH[r_e] + k_slot) * P + lane_e
    for c in range(NCORES):
        m = (core_e == c) & to_lo
        slots_lo[c, pos_lo[m]] = s[m]
        m = (core_e == c) & ~to_lo
        slots_hi[c, pos_hi[m]] = s[m] - hi_base

    # call structure: split chunk streams into calls of <= J_MAX chunks
    def make_calls(total_chunks, cumK):
        calls = []  # (chunk_off, J, segments=[(r, j_off, j_len)])
        off = 0
        while off < total_chunks:
            jn = min(J_MAX, total_chunks - off)
            segs = []
            r0 = int(np.searchsorted(cumK, off, side="right")) - 1
            j = 0
            while j < jn:
                while int(cumK[r0 + 1]) <= off + j:
                    r0 += 1
                r_end = int(cumK[r0 + 1])
                seg_len = min(jn - j, r_end - (off + j))
                segs.append((r0, j, seg_len))
                j += seg_len
            calls.append((off, jn, segs))
            off += jn
        return calls

    calls_lo = make_calls(CL, cumKL)
    calls_hi = make_calls(CH, cumKH)

    # packed int16 index data: slot i of a call -> [16*(rep), i%16, i//16]
    def pack(slots):  # [NCORES, n_slots] -> [NCORES, 128, n_slots//16]
        ncols = slots.shape[1] // 16
        a = slots.reshape(NCORES, ncols, 16).transpose(0, 2, 1)  # [NC,16,cols]
        a = a.astype(np.uint16).view(np.int16)
        return np.tile(a, (1, 8, 1))  # replicate to 128 partitions

    idx_lo = pack(slots_lo) if CL else np.zeros((NCORES, 128, 0), np.int16)
    idx_hi = pack(slots_hi) if CH else np.zeros((NCORES, 128, 0), np.int16)
    idx_all = np.concatenate([idx_lo, idx_hi], axis=2)
    idx_all = np.ascontiguousarray(idx_all)
    lo_cols = idx_lo.shape[2]

    # padded, permuted, transposed x
    x = np.asarray(x, dtype=np.float32)
    x_pad = np.zeros((npad, x.shape[1]), dtype=np.float32)
    x_pad[new_id[:n_real]] = x
    xT = np.ascontiguousarray(
        x_pad.reshape(NCORES, npc, x.shape[1]).transpose(0, 2, 1)
    )  # [NCORES, F, npc]

    return dict(
        npad=npad, npc=npc, tpc=tpc, table_rows=table_rows,
        hi_base=hi_base, lo_top=lo_top,
        KL=KL.astype(int), KH=KH.astype(int),
        calls_lo=calls_lo, calls_hi=calls_hi,
        idx_all=idx_all, lo_cols=lo_cols,
        xT=xT, new_id=new_id, n_real=n_real,
        slots_lo=slots_lo, slots_hi=slots_hi, cumKL=cumKL, cumKH=cumKH,
    )


def _wfull(W, a_src, a_dst):
    W = np.asarray(W, dtype=np.float32)
    fin = W.shape[0]
    Wf = W.reshape(fin, HC)
    Was = np.zeros((HC, H), dtype=np.float32)
    Wad = np.zeros((HC, H), dtype=np.float32)
    for h in range(H):
        Was[h * C:(h + 1) * C, h] = np.asarray(a_src, np.float32)[h]
        Wad[h * C:(h + 1) * C, h] = np.asarray(a_dst, np.float32)[h]
    return np.ascontiguousarray(
        np.concatenate([Wf, Wf @ Was, Wf @ Wad], axis=1)
    )  # [fin, 136]


def _dummy_row():
    row = np.zeros(256, dtype=np.uint16)
    fpart = np.array([-1e30] * 4 + [0.0] * 4, dtype=np.float32)
    row[128:144] = fpart.view(np.uint16)
    return row[None, :]


# ---------------------------------------------------------------------------
# device program
# ---------------------------------------------------------------------------

def _build(st):
    npc, tpc = st["npc"], st["tpc"]
    table_rows = st["table_rows"]
    hi_base, lo_top = st["hi_base"], st["lo_top"]
    KL, KH = st["KL"], st["KH"]
    calls_lo, calls_hi = st["calls_lo"], st["calls_hi"]
    tot_cols = st["idx_all"].shape[2]
    lo_cols = st["lo_cols"]
    npad = st["npad"]

    nc = bacc.Bacc(None, target_bir_lowering=False)

    xT_in = nc.dram_tensor("xT", [F_IN, npc], f32, kind="ExternalInput")
    idx_in = nc.dram_tensor("idx_all", [128, max(tot_cols, 16)], i16, kind="ExternalInput")
    wfull1_in = nc.dram_tensor("wfull1", [F_IN, 136], f32, kind="ExternalInput")
    wfull2_in = nc.dram_tensor("wfull2", [HC, 136], f32, kind="ExternalInput")
    wc_in = nc.dram_tensor("wc", [HC, NCLS], f32, kind="ExternalInput")
    b1_in = nc.dram_tensor("b1", [1, HC], f32, kind="ExternalInput")
    b2_in = nc.dram_tensor("b2", [1, HC], f32, kind="ExternalInput")
    bc_in = nc.dram_tensor("bc", [1, NCLS], f32, kind="ExternalInput")
    dummy_in = nc.dram_tensor("dummyrow", [1, 256], u16, kind="ExternalInput")
    ident16_in = nc.dram_tensor("ident16", [P, P], f16, kind="ExternalInput")
    ident32_in = nc.dram_tensor("ident32", [P, P], f32, kind="ExternalInput")

    logits_out = nc.dram_tensor("logits", [npc, NCLS], f32, kind="ExternalOutput")

    t_local = nc.dram_tensor("t_local", [npc, 256], u16)
    t_full = nc.dram_tensor("t_full", [table_rows, 256], u16, addr_space="Shared")
    x2t_dram = nc.dram_tensor("x2t", [HC, npc], f32)

    rg = [list(range(NCORES))]

    with tile.TileContext(nc) as tc:
        with (
            tc.tile_pool(name="const", bufs=1) as constp,
            tc.tile_pool(name="xt", bufs=1) as xtp,
            tc.tile_pool(name="wf", bufs=2) as wfp,
            tc.tile_pool(name="ald", bufs=2) as aldp,
            tc.tile_pool(name="tt", bufs=3) as ttp,
            tc.tile_pool(name="g", bufs=2) as gp,
            tc.tile_pool(name="m", bufs=2) as mp,
            tc.tile_pool(name="w32", bufs=2) as wp,
            tc.tile_pool(name="acc", bufs=tpc) as accp,
            tc.tile_pool(name="norm", bufs=3) as normp,
            tc.tile_pool(name="smalls", bufs=4) as smallp,
            tc.tile_pool(name="psd", bufs=2, space="PSUM") as psd,
            tc.tile_pool(name="pse", bufs=3, space="PSUM") as pse,
            tc.tile_pool(name="pst", bufs=2, space="PSUM") as pstp,
            tc.tile_pool(name="psc", bufs=1, space="PSUM") as pscp,
        ):
            # constants
            ident16 = constp.tile([P, P], f16)
            nc.sync.dma_start(out=ident16[:, :], in_=ident16_in[:, :])
            ident32 = constp.tile([P, P], f32)
            nc.sync.dma_start(out=ident32[:, :], in_=ident32_in[:, :])
            wc_sb = constp.tile([HC, NCLS], f32)
            nc.sync.dma_start(out=wc_sb[:, :], in_=wc_in[:, :])
            dummy_sb = constp.tile([1, 256], u16)
            nc.sync.dma_start(out=dummy_sb[:, :], in_=dummy_in[:, :])
            idx_sb = constp.tile([128, max(tot_cols, 16)], i16)
            nc.sync.dma_start(out=idx_sb[:, :], in_=idx_in[:, :])
            b_sb = {}
            for name, hnd, w in (("b1", b1_in, HC), ("b2", b2_in, HC), ("bc", bc_in, NCLS)):
                t = constp.tile([P, w], f32, name=f"bsb_{name}", tag=f"bsb_{name}")
                nc.sync.dma_start(out=t[:, :], in_=hnd[0:1, :].to_broadcast((P, w)))
                b_sb[name] = t
            # write hi-dummy table row once (outside AG range)
            if hi_base:
                nc.sync.dma_start(out=t_full[npad:npad + 1, :], in_=dummy_sb[:, :])

            for layer in (1, 2):
                wf_sb = wfp.tile([F_IN, 136], f32)
                nc.sync.dma_start(
                    out=wf_sb[:, :], in_=(wfull1_in if layer == 1 else wfull2_in)[:, :]
                )
                xt_sb = xtp.tile([F_IN, npc], f32, tag="xt")
                nc.sync.dma_start(
                    out=xt_sb[:, :],
                    in_=(xT_in[:, :] if layer == 1 else x2t_dram[:, :]),
                )

                # ---- dense phase: node table ----
                for r in range(tpc):
                    ps = psd.tile([P, 136], f32)
                    nc.tensor.matmul(
                        out=ps[:, :],
                        lhsT=xt_sb[:, r * P:(r + 1) * P],
                        rhs=wf_sb[:, :],
                        start=True, stop=True,
                    )
                    tt = ttp.tile([P, 256], u16, tag="tt")
                    nc.vector.memset(tt[:, 144:256], 0)
                    nc.vector.tensor_copy(
                        out=tt[:, 0:128].bitcast(f16), in_=ps[:, 0:128]
                    )
                    nc.vector.tensor_copy(
                        out=tt[:, 128:144].bitcast(f32), in_=ps[:, 128:136]
                    )
                    nc.sync.dma_start(
                        out=t_local[r * P:(r + 1) * P, :], in_=tt[:, :]
                    )

                # ald prefetch from local table (own rows == own dst tiles)
                ald_sb = aldp.tile([P, tpc, 8], u16, tag="ald")
                nc.sync.dma_start(
                    out=ald_sb[:, :, :],
                    in_=t_local[:, 136:144].rearrange("(r l) w -> l r w", l=P),
                )

                # ---- AllGather the table ----
                nc.gpsimd.collective_compute(
                    "AllGather",
                    mybir.AluOpType.bypass,
                    ins=[t_local[:, :]],
                    outs=[t_full[0:npad, :]],
                    replica_groups=rg,
                )
                # lo-dummy row (row 0) must have als = -1e30
                nc.sync.dma_start(out=t_full[0:1, :], in_=dummy_sb[:, :])

                # ---- edge phase ----
                acc = [accp.tile([P, 132], f32, tag="acc", name=f"acc{i}")
                       for i in range(tpc)]
                for a in acc:
                    nc.vector.memset(a[:, :], 0.0)

                ald_f32 = ald_sb[:, :, :].bitcast(f32)  # [P, tpc, 4]

                for wname, calls, col0, base, wtop in (
                    ("lo", calls_lo, 0, 0, lo_top),
                    ("hi", calls_hi, lo_cols, hi_base, table_rows),
                ):
                    if base == 0 and wname == "hi":
                        continue
                    win_rows = wtop - base
                    for (chunk_off, jn, segs) in calls:
                        g = gp.tile([P, jn, 256], u16, tag="g")
                        nc.gpsimd.dma_gather(
                            out_ap=g[:, :, :],
                            in_ap=t_full[base:base + win_rows, :],
                            idxs_ap=idx_sb[:, col0 + chunk_off * 8:
                                           col0 + (chunk_off + jn) * 8],
                            num_idxs=jn * P,
                            num_idxs_reg=jn * P,
                            elem_size=256,
                        )
                        w32 = wp.tile([P, jn, 4], f32, tag="w32")
                        for (r, j0, jl) in segs:
                            nc.vector.tensor_tensor(
                                out=w32[:, j0:j0 + jl, :],
                                in0=g[:, j0:j0 + jl, 128:136].bitcast(f32),
                                in1=ald_f32[:, r:r + 1, :].to_broadcast((P, jl, 4)),
                                op=mybir.AluOpType.add,
                            )
                        wtmp = wp.tile([P, jn, 4], f32, tag="wtmp")
                        nc.vector.tensor_scalar_mul(
                            out=wtmp[:, :, :], in0=w32[:, :, :], scalar1=NEG
                        )
                        nc.vector.tensor_tensor(
                            out=w32[:, :, :], in0=w32[:, :, :], in1=wtmp[:, :, :],
                            op=mybir.AluOpType.max,
                        )
                        nc.scalar.activation(
                            out=w32[:, :, :], in_=w32[:, :, :],
                            func=mybir.ActivationFunctionType.Exp,
                        )
                        m = mp.tile([P, jn, 132], f16, tag="m")
                        nc.vector.tensor_copy(out=m[:, :, 128:132], in_=w32[:, :, :])
                        nc.vector.tensor_tensor(
                            out=m[:, :, 0:128].rearrange("p j (h c) -> p j h c", h=H),
                            in0=g[:, :, 0:128].bitcast(f16).rearrange(
                                "p j (h c) -> p j h c", h=H),
                            in1=m[:, :, 128:132][:, :, :, None].to_broadcast(
                                (P, jn, H, C)),
                            op=mybir.AluOpType.mult,
                        )
                        for (r, j0, jl) in segs:
                            ps = pse.tile([P, 132], f32, tag="pse")
                            for j in range(j0, j0 + jl):
                                nc.tensor.matmul(
                                    out=ps[:, :],
                                    lhsT=ident16[:, :],
                                    rhs=m[:, j, :],
                                    start=(j == j0), stop=(j == j0 + jl - 1),
                                )
                            nc.vector.tensor_tensor(
                                out=acc[r][:, :], in0=acc[r][:, :], in1=ps[:, :],
                                op=mybir.AluOpType.add,
                            )

                # ---- normalize + activation + tail ----
                bias = b_sb["b1"] if layer == 1 else b_sb["b2"]
                for r in range(tpc):
                    recip = smallp.tile([P, 4], f32, tag="recip")
                    nc.vector.tensor_scalar_add(
                        out=recip[:, :], in0=acc[r][:, 128:132], scalar1=EPS
                    )
                    nc.vector.reciprocal(out=recip[:, :], in_=recip[:, :])
                    xn = normp.tile([P, HC], f32, tag="xn")
                    nc.vector.tensor_tensor(
                        out=xn[:, :].rearrange("p (h c) -> p h c", h=H),
                        in0=acc[r][:, 0:128].rearrange("p (h c) -> p h c", h=H),
                        in1=recip[:, :, None].to_broadcast((P, H, C)),
                        op=mybir.AluOpType.mult,
                    )
                    nc.vector.tensor_tensor(
                        out=xn[:, :], in0=xn[:, :], in1=bias[:, :],
                        op=mybir.AluOpType.add,
                    )
                    xtmp = normp.tile([P, HC], f32, tag="xtmp")
                    nc.vector.tensor_scalar_mul(
                        out=xtmp[:, :], in0=xn[:, :], scalar1=NEG
                    )
                    nc.vector.tensor_tensor(
                        out=xn[:, :], in0=xn[:, :], in1=xtmp[:, :],
                        op=mybir.AluOpType.max,
                    )
                    pt = pstp.tile([P, P], f32, tag="pt")
                    nc.tensor.transpose(
                        out=pt[:, :], in_=xn[:, :], identity=ident32[:, :]
                    )
                    xt2 = ttp.tile([P, P], f32, tag="xt2")
                    nc.vector.tensor_copy(out=xt2[:, :], in_=pt[:, :])
                    if layer == 1:
                        nc.sync.dma_start(
                            out=x2t_dram[:, r * P:(r + 1) * P], in_=xt2[:, :]
                        )
                    else:
                        pc = pscp.tile([P, NCLS], f32, tag="pc")
                        nc.tensor.matmul(
                            out=pc[:, :], lhsT=xt2[:, :], rhs=wc_sb[:, :],
                            start=True, stop=True,
                        )
                        lg = normp.tile([P, NCLS], f32, tag="lg")
                        nc.vector.tensor_tensor(
                            out=lg[:, :], in0=pc[:, :], in1=b_sb["bc"][:, :],
                            op=mybir.AluOpType.add,
                        )
                        mx = smallp.tile([P, 1], f32, tag="mx")
                        nc.vector.reduce_max(
                            out=mx[:, :], in_=lg[:, :], axis=mybir.AxisListType.X
                        )
                        zs = normp.tile([P, NCLS], f32, tag="zs")
                        nc.vector.tensor_scalar(
                            out=zs[:, :], in0=lg[:, :], scalar1=mx[:, :],
                            scalar2=None, op0=mybir.AluOpType.subtract,
                        )
                        es = normp.tile([P, NCLS], f32, tag="es")
                        nc.scalar.activation(
                            out=es[:, :], in_=zs[:, :],
                            func=mybir.ActivationFunctionType.Exp,
                        )
                        sm = smallp.tile([P, 1], f32, tag="sm")
                        nc.vector.reduce_sum(
                            out=sm[:, :], in_=es[:, :], axis=mybir.AxisListType.X
                        )
                        ls = smallp.tile([P, 1], f32, tag="ls")
                        nc.scalar.activation(
                            out=ls[:, :], in_=sm[:, :],
                            func=mybir.ActivationFunctionType.Ln,
                        )
                        ot = normp.tile([P, NCLS], f32, tag="ot")
                        nc.vector.tensor_scalar(
                            out=ot[:, :], in0=zs[:, :], scalar1=ls[:, :],
                            scalar2=None, op0=mybir.AluOpType.subtract,
                        )
                        nc.sync.dma_start(
                            out=logits_out[r * P:(r + 1) * P, :], in_=ot[:, :]
                        )

    nc.finalize()
    return nc


# ---------------------------------------------------------------------------
# entry point
# ---------------------------------------------------------------------------

_CACHE = {}


def kernel(x, edge_index, W1, a1_src, a1_dst, b1, W2, a2_src, a2_dst, b2, Wc, bc):
    global LAST_EXEC_NS
    import os

    x = np.asarray(x, dtype=np.float32)
    n_real = x.shape[0]
    ekey = hash((n_real,) + tuple(np.asarray(edge_index[0][:16]).tolist())
                ) ^ hash(np.asarray(edge_index).tobytes())
    if ekey in _CACHE:
        nc, st = _CACHE[ekey]
    else:
        st = _preprocess(x, edge_index, n_real)
        nc = _build(st)
        _CACHE[ekey] = (nc, st)
        st["xT"] = st["xT"]  # keep
    # per-run inputs (x could differ even with same edges; recompute xT)
    st2 = st
    npad, npc = st2["npad"], st2["npc"]
    new_id = st2["new_id"]
    x_pad = np.zeros((npad, F_IN), dtype=np.float32)
    x_pad[new_id[:n_real]] = x
    xT = np.ascontiguousarray(
        x_pad.reshape(NCORES, npc, F_IN).transpose(0, 2, 1))

    wfull1 = _wfull(W1, a1_src, a1_dst)
    wfull2 = _wfull(W2, a2_src, a2_dst)
    wc = np.ascontiguousarray(np.asarray(Wc, dtype=np.float32))
    b1r = np.asarray(b1, dtype=np.float32)[None, :]
    b2r = np.asarray(b2, dtype=np.float32)[None, :]
    bcr = np.asarray(bc, dtype=np.float32)[None, :]
    dummy = _dummy_row()
    ident16 = np.eye(P, dtype=np.float16)
    ident32 = np.eye(P, dtype=np.float32)

    idx_all = st2["idx_all"]
    if idx_all.shape[2] == 0:
        idx_all = np.zeros((NCORES, 128, 16), np.int16)
    elif idx_all.shape[2] < 16:
        pad = np.zeros((NCORES, 128, 16 - idx_all.shape[2]), np.int16)
        idx_all = np.concatenate([idx_all, pad], axis=2)

    in_maps = []
    for c in range(NCORES):
        in_maps.append({
            "xT": xT[c],
            "idx_all": np.ascontiguousarray(idx_all[c]),
            "wfull1": wfull1, "wfull2": wfull2, "wc": wc,
            "b1": b1r, "b2": b2r, "bc": bcr,
            "dummyrow": dummy, "ident16": ident16, "ident32": ident32,
        })

    os.environ.setdefault("BASS_NEVER_TRACE", "1")  # no NTFF hook in this env
    res = run_bass_kernel_spmd(nc, in_maps, core_ids=list(range(NCORES)))
    LAST_EXEC_NS = res.exec_time_ns

    logits_pad = np.concatenate([res.results[c]["logits"] for c in range(NCORES)], axis=0)
    return logits_pad[new_id[:n_real]].astype(np.float32)



# revision 2
# speedup vs baseline: 1.1307x; 1.1307x over previous
"""Trainium2 Bass kernel for 2-layer GAT node classification (50K nodes, 800K edges).

v2 strategy (vs baseline):
  - Layer 1 needs NO collective: x is a full input on every core, so each core
    computes the FULL node table (replicated dense phase, 392 tiles) locally.
    Layer 2 uses a single strided AllGather that moves only the 272B payload
    of each 512B table row.
  - Per-core node order is ROTATED (own core's block first) so the SPMD
    program can extract own-destination data at fixed positions. Layer-1
    gather indices are in rotated order; layer-2 indices in global order
    (the AllGather produces rank-major = global order).
  - Edge phase is per-destination-round: one PSUM accumulator per round,
    one dma_gather per (round, window), leaky-relu/exp on the Activation
    engine (Prelu+Exp+Ln+Copy share one table -> single table load),
    alpha duplicated into pairs so the message multiply hits the DVE 2x mode.
  - Normalization fused per round; layer-1 rounds immediately produce the
    layer-2 table rows (transpose + matmul). log_softmax batches all Ln calls
    into one.
"""
import sys

sys.path.insert(0, "/opt/trn_rl_repo")

import numpy as np

import concourse.bacc as bacc
import concourse.tile as tile
import concourse.mybir as mybir
from concourse.bass_utils import run_bass_kernel_spmd

P = 128
NCORES = 8
F_IN = 128
H = 4
C = 32
HC = 128
NCLS = 40
NEG = 0.2
WIN = 32768
RING = 16384  # default SWDGE ring; larger values hang the device
MAXCH = (RING // 16) // P  # max chunks per dma_gather call

f32 = mybir.dt.float32
f16 = mybir.dt.float16
u16 = mybir.dt.uint16
i16 = mybir.dt.int16

LAST_EXEC_NS = None
import os as _os
NO_PRELU = _os.environ.get("V2_NO_PRELU", "0") == "1"
NO_PAIR = _os.environ.get("V2_NO_PAIR", "0") == "1"
NO_LN = _os.environ.get("V2_NO_LN", "0") == "1"
NO_GATHER = _os.environ.get("V2_NO_GATHER", "0") == "1"
NO_COLL = _os.environ.get("V2_NO_COLL", "0") == "1"



# ---------------------------------------------------------------------------
# host preprocessing
# ---------------------------------------------------------------------------

def _cumcount(keys):
    n = len(keys)
    if n == 0:
        return np.zeros(0, dtype=np.int64)
    first = np.ones(n, dtype=bool)
    first[1:] = keys[1:] != keys[:-1]
    idx = np.arange(n)
    start = np.maximum.accumulate(np.where(first, idx, 0))
    return idx - start


def _build_grids(src_row, dst_newid, npc, tpc, table_rows):
    """Per-core slot grids for one layer.

    src_row: [Etot, NCORES] table row of the source as seen by each core
             (layer 1: rotated; layer 2: same global row for all cores).
    dst_newid: [Etot] global new id of the destination.
    Returns KL, KH [tpc] (common across cores) and per-core packed slot
    arrays (values = window-relative table rows).
    """
    hi_base = max(0, table_rows - WIN)
    dst_core = dst_newid // npc
    r_e = (dst_newid % npc) // P
    lane_e = dst_newid % P

    kl_counts = np.zeros((NCORES, tpc, P), dtype=np.int64)
    kh_counts = np.zeros((NCORES, tpc, P), dtype=np.int64)
    per_core = []
    for c in range(NCORES):
        m = dst_core == c
        rows = src_row[m, c] if src_row.ndim == 2 else src_row[m]
        d_r = r_e[m]
        d_lane = lane_e[m]
        cat = np.full(len(rows), 2, dtype=np.int8)  # flex
        cat[rows < hi_base] = 0  # lo only
        cat[rows >= WIN] = 1     # hi only
        dkey = d_r * P + d_lane
        o = np.argsort(dkey, kind="stable")
        rows, d_r, d_lane, cat, dkey = rows[o], d_r[o], d_lane[o], cat[o], dkey[o]
        ndeg = np.bincount(dkey, minlength=tpc * P)
        nlo = np.bincount(dkey[cat == 0], minlength=tpc * P)
        nhi = np.bincount(dkey[cat == 1], minlength=tpc * P)
        kl_node = np.maximum(nlo, np.minimum(ndeg - nhi, (ndeg + 1) // 2))
        flex_rank = np.zeros(len(rows), dtype=np.int64)
        mflex = cat == 2
        flex_rank[mflex] = _cumcount(dkey[mflex])
        to_lo = (cat == 0) | (mflex & (flex_rank < (kl_node - nlo)[dkey]))
        k_slot = np.zeros(len(rows), dtype=np.int64)
        for mm in (to_lo, ~to_lo):
            k_slot[mm] = _cumcount(dkey[mm])
        kl_counts[c] = kl_node.reshape(tpc, P)
        kh_counts[c] = (ndeg - kl_node).reshape(tpc, P)
        per_core.append((rows, d_r, d_lane, to_lo, k_slot))

    KL = kl_counts.max(axis=(0, 2)).astype(np.int64)
    KH = kh_counts.max(axis=(0, 2)).astype(np.int64)
    cumKL = np.concatenate([[0], np.cumsum(KL)])
    cumKH = np.concatenate([[0], np.cumsum(KH)])
    CL, CH = int(cumKL[-1]), int(cumKH[-1])

    DUM_LO = 0
    DUM_HI = table_rows - 1 - hi_base
    slots_lo = np.full((NCORES, CL * P), DUM_LO, dtype=np.int64)
    slots_hi = np.full((NCORES, CH * P), DUM_HI, dtype=np.int64)
    for c in range(NCORES):
        rows, d_r, d_lane, to_lo, k_slot = per_core[c]
        pos_lo = (cumKL[d_r] + k_slot) * P + d_lane
        pos_hi = (cumKH[d_r] + k_slot) * P + d_lane
        slots_lo[c, pos_lo[to_lo]] = rows[to_lo]
        slots_hi[c, pos_hi[~to_lo]] = rows[~to_lo] - hi_base
    return KL, KH, slots_lo, slots_hi


def _pack(slots):
    """[NCORES, n_slots] -> [NCORES, 128, n_slots//16] int16 idx layout."""
    ncols = slots.shape[1] // 16
    if ncols == 0:
        return np.zeros((NCORES, 128, 0), np.int16)
    a = slots.reshape(NCORES, ncols, 16).transpose(0, 2, 1)
    a = a.astype(np.uint16).view(np.int16)
    return np.tile(a, (1, 8, 1))


def _preprocess(x, edge_index, n_real):
    n_tiles = -(-(n_real + 1) // P)
    n_tiles = -(-n_tiles // NCORES) * NCORES
    npad = n_tiles * P
    tpc = n_tiles // NCORES
    npc = tpc * P
    table_rows = 128 + npad + 128  # dummy tile, nodes, dummy tile
    assert table_rows <= 2 * WIN, "two int16 windows must cover the table"

    src0 = np.asarray(edge_index[0]).astype(np.int64)
    dst0 = np.asarray(edge_index[1]).astype(np.int64)

    deg = np.bincount(dst0, minlength=npad).astype(np.int64)
    deg[:n_real] += 1
    order = np.argsort(deg, kind="stable")
    pos = np.empty(npad, dtype=np.int64)
    pos[order] = np.arange(npad)
    tile_of = pos // P
    lane_of = pos % P
    r_of = tile_of // NCORES
    c_of = tile_of % NCORES
    new_id = c_of * npc + r_of * P + lane_of  # old -> global new id

    all_src = np.concatenate([new_id[src0], new_id[:n_real]])
    all_dst = np.concatenate([new_id[dst0], new_id[:n_real]])

    # layer-1 source rows: rotated per core (own block first), +128 offset
    blk = all_src // npc
    within = all_src % npc
    rot_rows = np.empty((len(all_src), NCORES), dtype=np.int64)
    for c in range(NCORES):
        rot_rows[:, c] = 128 + ((blk - c) % NCORES) * npc + within
    KL1, KH1, sl1, sh1 = _build_grids(rot_rows, all_dst, npc, tpc, table_rows)
    # layer-2 source rows: global order
    KL2, KH2, sl2, sh2 = _build_grids(
        128 + all_src, all_dst, npc, tpc, table_rows
    )

    idx1 = np.concatenate([_pack(sl1), _pack(sh1)], axis=2)
    idx2 = np.concatenate([_pack(sl2), _pack(sh2)], axis=2)

    return dict(
        npad=npad, npc=npc, tpc=tpc, table_rows=table_rows,
        KL1=KL1, KH1=KH1, KL2=KL2, KH2=KH2,
        idx1=np.ascontiguousarray(idx1), idx2=np.ascontiguousarray(idx2),
        new_id=new_id, n_real=n_real,
    )


def _wfull(W, a_src, a_dst):
    W = np.asarray(W, dtype=np.float32)
    fin = W.shape[0]
    Wf = W.reshape(fin, HC)
    Was = np.zeros((HC, H), dtype=np.float32)
    Wad = np.zeros((HC, H), dtype=np.float32)
    for h in range(H):
        Was[h * C:(h + 1) * C, h] = np.asarray(a_src, np.float32)[h]
        Wad[h * C:(h + 1) * C, h] = np.asarray(a_dst, np.float32)[h]
    out = np.concatenate([Wf, Wf @ Was, Wf @ Wad], axis=1)  # [fin, 136]
    return np.ascontiguousarray(out.astype(np.float16))


def _dummy_row():
    row = np.zeros(256, dtype=np.uint16)
    fpart = np.array([-1e30] * 4 + [0.0] * 4, dtype=np.float32)
    row[128:144] = fpart.view(np.uint16)
    return row[None, :]


# ---------------------------------------------------------------------------
# device program
# ---------------------------------------------------------------------------

def _build(st, b1_zero, b2_zero):
    npc, tpc = st["npc"], st["tpc"]
    npad = st["npad"]
    table_rows = st["table_rows"]
    hi_base = max(0, table_rows - WIN)
    n_tiles = npad // P
    KLs = {1: st["KL1"], 2: st["KL2"]}
    KHs = {1: st["KH1"], 2: st["KH2"]}
    ncols = {1: st["idx1"].shape[2], 2: st["idx2"].shape[2]}
    locol = {
        1: 8 * int(st["KL1"].sum()),
        2: 8 * int(st["KL2"].sum()),
    }
    Kmax = max(
        int((st["KL1"] + st["KH1"]).max()), int((st["KL2"] + st["KH2"]).max())
    )
    idxcols = max(ncols[1], ncols[2], 16)

    nc = bacc.Bacc(None, target_bir_lowering=False,
                   dynamic_dma_scratch_size=RING)

    xT_in = nc.dram_tensor("xT", [F_IN, npad], f16, kind="ExternalInput")
    idx1_in = nc.dram_tensor("idx1", [128, idxcols], i16, kind="ExternalInput")
    idx2_in = nc.dram_tensor("idx2", [128, idxcols], i16, kind="ExternalInput")
    wfull1_in = nc.dram_tensor("wfull1", [F_IN, 136], f16, kind="ExternalInput")
    wfull2_in = nc.dram_tensor("wfull2", [HC, 136], f16, kind="ExternalInput")
    wc_in = nc.dram_tensor("wc", [HC, NCLS], f16, kind="ExternalInput")
    b1_in = nc.dram_tensor("b1", [1, HC], f32, kind="ExternalInput")
    b2_in = nc.dram_tensor("b2", [1, HC], f32, kind="ExternalInput")
    bc_in = nc.dram_tensor("bc", [1, NCLS], f32, kind="ExternalInput")
    dummy_in = nc.dram_tensor("dummyrow", [1, 256], u16, kind="ExternalInput")
    ident_in = nc.dram_tensor("ident16", [P, P], f16, kind="ExternalInput")

    logits_out = nc.dram_tensor("logits", [npc, NCLS], f32, kind="ExternalOutput")

    t_full1 = nc.dram_tensor("t_full1", [table_rows, 256], u16)
    t2_local = nc.dram_tensor("t2_local", [npc, 256], u16)
    t_full2 = nc.dram_tensor("t_full2", [table_rows, 256], u16, addr_space="Shared")

    rg = [list(range(NCORES))]

    with tile.TileContext(nc) as tc:
        with (
            tc.tile_pool(name="const", bufs=1) as constp,
            tc.tile_pool(name="xt", bufs=3) as xtp,
            tc.tile_pool(name="tstage", bufs=3) as tsp,
            tc.tile_pool(name="idx", bufs=2) as idxp,
            tc.tile_pool(name="ald", bufs=1) as aldp,
            tc.tile_pool(name="g", bufs=3) as gp,
            tc.tile_pool(name="m", bufs=2) as mp,
            tc.tile_pool(name="w32", bufs=3) as wp,
            tc.tile_pool(name="norm", bufs=4) as normp,
            tc.tile_pool(name="small", bufs=6) as smallp,
            tc.tile_pool(name="hkeep", bufs=1) as hkp,
            tc.tile_pool(name="cls", bufs=1) as clsp,
            tc.tile_pool(name="psd", bufs=2, space="PSUM") as psd,
            tc.tile_pool(name="pse", bufs=2, space="PSUM") as pse,
            tc.tile_pool(name="pst", bufs=2, space="PSUM") as pstp,
            tc.tile_pool(name="psc", bufs=1, space="PSUM") as pscp,
        ):
            ACT = mybir.ActivationFunctionType
            ident = constp.tile([P, P], f16)
            nc.sync.dma_start(out=ident[:, :], in_=ident_in[:, :])
            wf1 = constp.tile([F_IN, 136], f16)
            nc.sync.dma_start(out=wf1[:, :], in_=wfull1_in[:, :])
            wf2 = constp.tile([HC, 136], f16)
            nc.sync.dma_start(out=wf2[:, :], in_=wfull2_in[:, :])
            wc_sb = constp.tile([HC, NCLS], f16)
            nc.sync.dma_start(out=wc_sb[:, :], in_=wc_in[:, :])
            dummy_sb = constp.tile([1, 256], u16)
            nc.sync.dma_start(out=dummy_sb[:, :], in_=dummy_in[:, :])
            b_sb = {}
            for name, hnd, w in (("b1", b1_in, HC), ("b2", b2_in, HC),
                                 ("bc", bc_in, NCLS)):
                t = constp.tile([P, w], f32, name=f"bsb_{name}", tag=f"bsb_{name}")
                nc.sync.dma_start(out=t[:, :], in_=hnd[0:1, :].to_broadcast((P, w)))
                b_sb[name] = t
            bf16_1 = constp.tile([P, HC], f16, name="b1f16", tag="b1f16")
            nc.vector.tensor_copy(out=bf16_1[:, :], in_=b_sb["b1"][:, :])
            bf16_2 = constp.tile([P, HC], f16, name="b2f16", tag="b2f16")
            nc.vector.tensor_copy(out=bf16_2[:, :], in_=b_sb["b2"][:, :])

            # dummy rows of both tables
            for tf in (t_full1, t_full2):
                nc.sync.dma_start(out=tf[0:1, :], in_=dummy_sb[:, :])
                nc.sync.dma_start(
                    out=tf[table_rows - 1:table_rows, :], in_=dummy_sb[:, :]
                )

            # ---- replicated dense phase for layer 1 ----
            ald1 = aldp.tile([P, tpc, 4], f32, name="ald1", tag="ald1")
            ald2 = aldp.tile([P, tpc, 4], f32, name="ald2", tag="ald2")
            BT = 3   # tiles per PSUM batch
            GB = 12  # tiles per DMA group
            for gg in range(0, n_tiles, GB):
                gcnt = min(GB, n_tiles - gg)
                xt = xtp.tile([P, gcnt * P], f16, tag="xt")
                nc.sync.dma_start(
                    out=xt[:, :], in_=xT_in[:, gg * P:(gg + gcnt) * P]
                )
                ts_ = tsp.tile([P, gcnt, 256], u16, tag="ts")
                for g0 in range(gg, gg + gcnt, BT):
                    gn = min(BT, gg + gcnt - g0)
                    o = g0 - gg
                    ps = psd.tile([P, gn, 136], f32, tag="psd")
                    for j in range(gn):
                        nc.tensor.matmul(
                            out=ps[:, j, :],
                            lhsT=xt[:, (o + j) * P:(o + j + 1) * P],
                            rhs=wf1[:, :],
                            start=True, stop=True,
                        )
                    nc.scalar.activation(
                        out=ts_[:, o:o + gn, 0:128].bitcast(f16),
                        in_=ps[:, :, 0:128],
                        func=ACT.Copy,
                    )
                    nc.vector.tensor_copy(
                        out=ts_[:, o:o + gn, 128:136].bitcast(f32),
                        in_=ps[:, :, 128:132]
                    )
                    if g0 < tpc:  # own tiles: extract a_dst . h
                        jn = min(gn, tpc - g0)
                        nc.vector.tensor_copy(
                            out=ald1[:, g0:g0 + jn, :], in_=ps[:, 0:jn, 132:136]
                        )
                nc.sync.dma_start(
                    out=t_full1[128 + gg * P:128 + (gg + gcnt) * P, 0:136].rearrange(
                        "(g p) w -> p g w", p=P),
                    in_=ts_[:, :, 0:136],
                )

            # ald for layer 2 is produced during layer-1 normalization.
            h1own = hkp.tile([P, tpc, HC], f16, name="h1own", tag="h1own")

            zsAll = clsp.tile([P, tpc, NCLS], f32, name="zsAll", tag="zsAll")
            smAll = clsp.tile([P, tpc], f32, name="smAll", tag="smAll")
            lnAll = clsp.tile([P, tpc], f32, name="lnAll", tag="lnAll")

            for layer in (1, 2):
                KL, KH = KLs[layer], KHs[layer]
                cumKL = np.concatenate([[0], np.cumsum(KL)])
                cumKH = np.concatenate([[0], np.cumsum(KH)])
                idx_sb = idxp.tile([128, idxcols], i16, tag="idx")
                nc.sync.dma_start(
                    out=idx_sb[:, :],
                    in_=(idx1_in if layer == 1 else idx2_in)[:, :],
                )
                t_full = t_full1 if layer == 1 else t_full2
                ald = ald1 if layer == 1 else ald2
                bias = bf16_1 if layer == 1 else bf16_2
                bzero = b1_zero if layer == 1 else b2_zero

                for r in range(tpc):
                    kl, kh = int(KL[r]), int(KH[r])
                    K = kl + kh
                    g = gp.tile([P, K, 256], u16, tag="g")
                    # lo gather
                    off = 0
                    col = 8 * int(cumKL[r])
                    while off < kl:
                        jn = min(MAXCH, kl - off)
                        nc.gpsimd.dma_gather(
                            out_ap=g[:, off:off + jn, :],
                            in_ap=t_full[0:min(WIN, table_rows), :],
                            idxs_ap=idx_sb[:, col:col + jn * 8],
                            num_idxs=jn * P,
                            num_idxs_reg=jn * P,
                            elem_size=256,
                        )
                        off += jn
                        col += jn * 8
                    # hi gather
                    off = 0
                    col = locol[layer] + 8 * int(cumKH[r])
                    while off < kh:
                        jn = min(MAXCH, kh - off)
                        nc.gpsimd.dma_gather(
                            out_ap=g[:, kl + off:kl + off + jn, :],
                            in_ap=t_full[hi_base:table_rows, :],
                            idxs_ap=idx_sb[:, col:col + jn * 8],
                            num_idxs=jn * P,
                            num_idxs_reg=jn * P,
                            elem_size=256,
                        )
                        off += jn
                        col += jn * 8

                    # w = exp(prelu(als_src + ald_dst))
                    w32 = wp.tile([P, K, 4], f32, tag="w32")
                    nc.vector.tensor_tensor(
                        out=w32[:, :, :],
                        in0=g[:, :, 128:136].bitcast(f32),
                        in1=ald[:, r:r + 1, :].to_broadcast((P, K, 4)),
                        op=mybir.AluOpType.add,
                    )
                    zt = wp.tile([P, K, 4], f32, tag="zt")
                    if NO_PRELU:
                        nc.vector.tensor_scalar_mul(
                            out=zt[:, :, :], in0=w32[:, :, :], scalar1=NEG,
                        )
                        nc.vector.tensor_tensor(
                            out=zt[:, :, :], in0=w32[:, :, :], in1=zt[:, :, :],
                            op=mybir.AluOpType.max,
                        )
                    else:
                        nc.scalar.activation(
                            out=zt[:, :, :], in_=w32[:, :, :],
                            func=ACT.Prelu, alpha=NEG,
                        )
                    m = mp.tile([P, K, 132], f16, tag="m")
                    nc.scalar.activation(
                        out=m[:, :, 128:132], in_=zt[:, :, :], func=ACT.Exp,
                    )
                    a2 = wp.tile([P, K, 4, 2], f16, tag="a2")
                    nc.vector.tensor_copy(
                        out=a2[:, :, :, :],
                        in_=m[:, :, 128:132][:, :, :, None].to_broadcast(
                            (P, K, 4, 2)),
                    )
                    nc.vector.tensor_tensor(
                        out=m[:, :, 0:128].rearrange(
                            "p k (h c d) -> p k h c d", h=H, d=2),
                        in0=g[:, :, 0:128].bitcast(f16).rearrange(
                            "p k (h c d) -> p k h c d", h=H, d=2),
                        in1=a2[:, :, :, None, :].to_broadcast((P, K, 4, 16, 2)),
                        op=mybir.AluOpType.mult,
                    )
                    # accumulate the round in PSUM via identity matmuls,
                    # two chunks per matmul (halves PE instruction count)
                    ps = pse.tile([P, 2, 132], f32, tag="pse")
                    if NO_PAIR:
                        nc.vector.memset(ps[:, 1, :], 0.0)
                        for j in range(K):
                            nc.tensor.matmul(
                                out=ps[:, 0, :], lhsT=ident[:, :], rhs=m[:, j, :],
                                start=(j == 0), stop=(j == K - 1),
                            )
                    else:
                        npairs = K // 2
                        for j in range(npairs):
                            nc.tensor.matmul(
                                out=ps[:, :, :], lhsT=ident[:, :],
                                rhs=m[:, 2 * j:2 * j + 2, :],
                                start=(j == 0),
                                stop=(j == npairs - 1 and K % 2 == 0),
                            )
                        if K % 2:
                            nc.tensor.matmul(
                                out=ps[:, 0, :], lhsT=ident[:, :],
                                rhs=m[:, K - 1, :],
                                start=False, stop=True,
                            )
                    acc0 = normp.tile([P, 132], f16, tag="acc0")
                    nc.scalar.activation(
                        out=acc0[:, :], in_=ps[:, 0, :], func=ACT.Copy,
                    )
                    accf = normp.tile([P, 132], f16, tag="accf")
                    nc.vector.tensor_tensor(
                        out=accf[:, :], in0=acc0[:, :], in1=ps[:, 1, :],
                        op=mybir.AluOpType.add,
                    )
                    # normalize + bias + leaky
                    den = smallp.tile([P, 4], f16, tag="den")
                    nc.vector.tensor_scalar_add(
                        out=den[:, :], in0=accf[:, 128:132], scalar1=1e-4,
                    )
                    rc2 = smallp.tile([P, 4, 2], f16, tag="rc2")
                    with nc.allow_low_precision(reason="f16 softmax denom"):
                        nc.vector.reciprocal(out=rc2[:, :, 0], in_=den[:, :])
                    nc.vector.tensor_copy(
                        out=rc2[:, :, 1], in_=rc2[:, :, 0],
                    )
                    xn = normp.tile([P, HC], f16, tag="xn")
                    nc.vector.tensor_tensor(
                        out=xn[:, :].rearrange("p (h c d) -> p h c d", h=H, d=2),
                        in0=accf[:, 0:128].rearrange(
                            "p (h c d) -> p h c d", h=H, d=2),
                        in1=rc2[:, :, None, :].to_broadcast((P, 4, 16, 2)),
                        op=mybir.AluOpType.mult,
                    )
                    if not bzero:
                        nc.vector.tensor_tensor(
                            out=xn[:, :], in0=xn[:, :], in1=bias[:, :],
                            op=mybir.AluOpType.add,
                        )
                    hnext = normp.tile([P, HC], f16, tag="hnext")
                    if NO_PRELU:
                        nc.vector.tensor_scalar_mul(
                            out=hnext[:, :], in0=xn[:, :], scalar1=NEG,
                        )
                        nc.vector.tensor_tensor(
                            out=hnext[:, :], in0=xn[:, :], in1=hnext[:, :],
                            op=mybir.AluOpType.max,
                        )
                    else:
                        nc.scalar.activation(
                            out=hnext[:, :], in_=xn[:, :], func=ACT.Prelu,
                            alpha=NEG,
                        )

                    ptr = pstp.tile([P, P], f16, tag="ptr")
                    nc.tensor.transpose(
                        out=ptr[:, :], in_=hnext[:, :], identity=ident[:, :]
                    )
                    hT = normp.tile([P, P], f16, tag="hT")
                    nc.scalar.activation(
                        out=hT[:, :], in_=ptr[:, :], func=ACT.Copy,
                    )
                    if layer == 1:
                        nc.vector.tensor_copy(
                            out=h1own[:, r, :], in_=hnext[:, :]
                        )
                        ps2 = psd.tile([P, 1, 136], f32, tag="psd")
                        nc.tensor.matmul(
                            out=ps2[:, 0, :], lhsT=hT[:, :], rhs=wf2[:, :],
                            start=True, stop=True,
                        )
                        t2s = tsp.tile([P, 1, 256], u16, tag="ts")
                        nc.scalar.activation(
                            out=t2s[:, 0, 0:128].bitcast(f16),
                            in_=ps2[:, 0, 0:128], func=ACT.Copy,
                        )
                        nc.vector.tensor_copy(
                            out=t2s[:, 0, 128:136].bitcast(f32),
                            in_=ps2[:, 0, 128:132],
                        )
                        nc.vector.tensor_copy(
                            out=ald2[:, r, :], in_=ps2[:, 0, 132:136]
                        )
                        nc.sync.dma_start(
                            out=t2_local[r * P:(r + 1) * P, 0:136],
                            in_=t2s[:, 0, 0:136],
                        )
                    else:
                        pc = pscp.tile([P, NCLS], f32, tag="pc")
                        nc.tensor.matmul(
                            out=pc[:, :], lhsT=hT[:, :], rhs=wc_sb[:, :],
                            start=True, stop=True,
                        )
                        lg = normp.tile([P, NCLS], f32, tag="lg")
                        nc.scalar.activation(
                            out=lg[:, :], in_=pc[:, :], func=ACT.Copy,
                        )
                        nc.vector.tensor_tensor(
                            out=lg[:, :], in0=lg[:, :], in1=b_sb["bc"][:, :],
                            op=mybir.AluOpType.add,
                        )
                        mx = smallp.tile([P, 1], f32, tag="mx")
                        nc.vector.reduce_max(
                            out=mx[:, :], in_=lg[:, :], axis=mybir.AxisListType.X
                        )
                        nc.vector.tensor_scalar(
                            out=zsAll[:, r, :], in0=lg[:, :], scalar1=mx[:, :],
                            scalar2=None, op0=mybir.AluOpType.subtract,
                        )
                        es = normp.tile([P, NCLS], f32, tag="lg2")
                        nc.scalar.activation(
                            out=es[:, :], in_=zsAll[:, r, :], func=ACT.Exp,
                        )
                        nc.vector.reduce_sum(
                            out=smAll[:, r:r + 1], in_=es[:, :],
                            axis=mybir.AxisListType.X,
                        )

                if layer == 1:
                    # distribute the layer-2 table (payload columns only)
                    nc.gpsimd.collective_compute(
                        "AllGather",
                        mybir.AluOpType.bypass,
                        ins=[t2_local[:, :]],
                        outs=[t_full2[128:128 + npad, :]],
                        replica_groups=rg,
                    )

            # ---- batched log-softmax tail ----
            nc.scalar.activation(
                out=lnAll[:, :], in_=smAll[:, :], func=ACT.Ln,
            )
            otAll = clsp.tile([P, tpc, NCLS], f32, name="otAll", tag="otAll")
            for r in range(tpc):
                nc.vector.tensor_scalar(
                    out=otAll[:, r, :], in0=zsAll[:, r, :],
                    scalar1=lnAll[:, r:r + 1], scalar2=None,
                    op0=mybir.AluOpType.subtract,
                )
            nc.sync.dma_start(
                out=logits_out[:, :].rearrange("(r l) c -> l r c", l=P),
                in_=otAll[:, :, :],
            )

    nc.finalize()
    return nc


# ---------------------------------------------------------------------------
# entry point
# ---------------------------------------------------------------------------

_CACHE = {}


def kernel(x, edge_index, W1, a1_src, a1_dst, b1, W2, a2_src, a2_dst, b2, Wc, bc):
    global LAST_EXEC_NS
    import os

    x = np.asarray(x, dtype=np.float32)
    n_real = x.shape[0]
    b1 = np.asarray(b1, dtype=np.float32)
    b2 = np.asarray(b2, dtype=np.float32)
    b1_zero = bool(np.all(b1 == 0))
    b2_zero = bool(np.all(b2 == 0))
    ekey = hash(np.asarray(edge_index).tobytes()) ^ hash((n_real, b1_zero, b2_zero))
    if ekey in _CACHE:
        nc, st = _CACHE[ekey]
    else:
        st = _preprocess(x, edge_index, n_real)
        nc = _build(st, b1_zero, b2_zero)
        _CACHE[ekey] = (nc, st)

    npad, npc, tpc = st["npad"], st["npc"], st["tpc"]
    new_id = st["new_id"]

    # per-core rotated xT (full table, f16)
    x_pad = np.zeros((npad, F_IN), dtype=np.float32)
    x_pad[new_id[:n_real]] = x
    x_blocks = x_pad.reshape(NCORES, npc, F_IN)

    wfull1 = _wfull(W1, a1_src, a1_dst)
    wfull2 = _wfull(W2, a2_src, a2_dst)
    wc = np.ascontiguousarray(np.asarray(Wc, dtype=np.float16))
    b1r = b1[None, :]
    b2r = b2[None, :]
    bcr = np.asarray(bc, dtype=np.float32)[None, :]
    dummy = _dummy_row()
    ident = np.eye(P, dtype=np.float16)

    idxcols = max(st["idx1"].shape[2], st["idx2"].shape[2], 16)

    def pad_idx(a):
        if a.shape[2] < idxcols:
            a = np.concatenate(
                [a, np.zeros((NCORES, 128, idxcols - a.shape[2]), np.int16)],
                axis=2)
        return a

    idx1 = pad_idx(st["idx1"])
    idx2 = pad_idx(st["idx2"])

    in_maps = []
    for c in range(NCORES):
        rot = np.roll(np.arange(NCORES), -c)  # own block first
        xT = np.ascontiguousarray(
            x_blocks[rot].reshape(npad, F_IN).T.astype(np.float16)
        )
        in_maps.append({
            "xT": xT,
            "idx1": np.ascontiguousarray(idx1[c]),
            "idx2": np.ascontiguousarray(idx2[c]),
            "wfull1": wfull1, "wfull2": wfull2, "wc": wc,
            "b1": b1r, "b2": b2r, "bc": bcr,
            "dummyrow": dummy, "ident16": ident,
        })

    os.environ.setdefault("BASS_NEVER_TRACE", "1")
    res = run_bass_kernel_spmd(nc, in_maps, core_ids=list(range(NCORES)))
    LAST_EXEC_NS = res.exec_time_ns

    logits_pad = np.concatenate(
        [res.results[c]["logits"] for c in range(NCORES)], axis=0
    )
    return logits_pad[new_id[:n_real]].astype(np.float32)


# revision 3
# speedup vs baseline: 1.1656x; 1.0309x over previous
"""Trainium2 Bass kernel for 2-layer GAT node classification (50K nodes, 800K edges).

v2 strategy (vs baseline):
  - Layer 1 needs NO collective: x is a full input on every core, so each core
    computes the FULL node table (replicated dense phase, 392 tiles) locally.
    Layer 2 uses a single strided AllGather that moves only the 272B payload
    of each 512B table row.
  - Per-core node order is ROTATED (own core's block first) so the SPMD
    program can extract own-destination data at fixed positions. Layer-1
    gather indices are in rotated order; layer-2 indices in global order
    (the AllGather produces rank-major = global order).
  - Edge phase is per-destination-round: one PSUM accumulator per round,
    one dma_gather per (round, window), leaky-relu/exp on the Activation
    engine (Prelu+Exp+Ln+Copy share one table -> single table load),
    alpha duplicated into pairs so the message multiply hits the DVE 2x mode.
  - Normalization fused per round; layer-1 rounds immediately produce the
    layer-2 table rows (transpose + matmul). log_softmax batches all Ln calls
    into one.
"""
import sys

sys.path.insert(0, "/opt/trn_rl_repo")

import numpy as np

import concourse.bacc as bacc
import concourse.tile as tile
import concourse.mybir as mybir
from concourse.bass_utils import run_bass_kernel_spmd

P = 128
NCORES = 8
F_IN = 128
H = 4
C = 32
HC = 128
NCLS = 40
NEG = 0.2
WIN = 32768
RING = 16384  # default SWDGE ring; larger values hang the device
MAXCH = (RING // 16) // P  # max chunks per dma_gather call

f32 = mybir.dt.float32
f16 = mybir.dt.float16
u16 = mybir.dt.uint16
i16 = mybir.dt.int16

LAST_EXEC_NS = None
import os as _os
NO_PRELU = _os.environ.get("V2_NO_PRELU", "0") == "1"
NO_PAIR = _os.environ.get("V2_NO_PAIR", "0") == "1"
NO_LN = _os.environ.get("V2_NO_LN", "0") == "1"
NO_GATHER = _os.environ.get("V2_NO_GATHER", "0") == "1"
NO_COLL = _os.environ.get("V2_NO_COLL", "0") == "1"



# ---------------------------------------------------------------------------
# host preprocessing
# ---------------------------------------------------------------------------

def _cumcount(keys):
    n = len(keys)
    if n == 0:
        return np.zeros(0, dtype=np.int64)
    first = np.ones(n, dtype=bool)
    first[1:] = keys[1:] != keys[:-1]
    idx = np.arange(n)
    start = np.maximum.accumulate(np.where(first, idx, 0))
    return idx - start


def _build_grids(src_row, dst_newid, npc, tpc, table_rows):
    """Per-core slot grids for one layer.

    src_row: [Etot, NCORES] table row of the source as seen by each core
             (layer 1: rotated; layer 2: same global row for all cores).
    dst_newid: [Etot] global new id of the destination.
    Returns KL, KH [tpc] (common across cores) and per-core packed slot
    arrays (values = window-relative table rows).
    """
    hi_base = max(0, table_rows - WIN)
    dst_core = dst_newid // npc
    r_e = (dst_newid % npc) // P
    lane_e = dst_newid % P

    kl_counts = np.zeros((NCORES, tpc, P), dtype=np.int64)
    kh_counts = np.zeros((NCORES, tpc, P), dtype=np.int64)
    per_core = []
    for c in range(NCORES):
        m = dst_core == c
        rows = src_row[m, c] if src_row.ndim == 2 else src_row[m]
        d_r = r_e[m]
        d_lane = lane_e[m]
        cat = np.full(len(rows), 2, dtype=np.int8)  # flex
        cat[rows < hi_base] = 0  # lo only
        cat[rows >= WIN] = 1     # hi only
        dkey = d_r * P + d_lane
        o = np.argsort(dkey, kind="stable")
        rows, d_r, d_lane, cat, dkey = rows[o], d_r[o], d_lane[o], cat[o], dkey[o]
        ndeg = np.bincount(dkey, minlength=tpc * P)
        nlo = np.bincount(dkey[cat == 0], minlength=tpc * P)
        nhi = np.bincount(dkey[cat == 1], minlength=tpc * P)
        kl_node = np.maximum(nlo, np.minimum(ndeg - nhi, (ndeg + 1) // 2))
        flex_rank = np.zeros(len(rows), dtype=np.int64)
        mflex = cat == 2
        flex_rank[mflex] = _cumcount(dkey[mflex])
        to_lo = (cat == 0) | (mflex & (flex_rank < (kl_node - nlo)[dkey]))
        k_slot = np.zeros(len(rows), dtype=np.int64)
        for mm in (to_lo, ~to_lo):
            k_slot[mm] = _cumcount(dkey[mm])
        kl_counts[c] = kl_node.reshape(tpc, P)
        kh_counts[c] = (ndeg - kl_node).reshape(tpc, P)
        per_core.append((rows, d_r, d_lane, to_lo, k_slot))

    KL = kl_counts.max(axis=(0, 2)).astype(np.int64)
    KH = kh_counts.max(axis=(0, 2)).astype(np.int64)
    cumKL = np.concatenate([[0], np.cumsum(KL)])
    cumKH = np.concatenate([[0], np.cumsum(KH)])
    CL, CH = int(cumKL[-1]), int(cumKH[-1])

    DUM_LO = 0
    DUM_HI = table_rows - 1 - hi_base
    slots_lo = np.full((NCORES, CL * P), DUM_LO, dtype=np.int64)
    slots_hi = np.full((NCORES, CH * P), DUM_HI, dtype=np.int64)
    for c in range(NCORES):
        rows, d_r, d_lane, to_lo, k_slot = per_core[c]
        pos_lo = (cumKL[d_r] + k_slot) * P + d_lane
        pos_hi = (cumKH[d_r] + k_slot) * P + d_lane
        slots_lo[c, pos_lo[to_lo]] = rows[to_lo]
        slots_hi[c, pos_hi[~to_lo]] = rows[~to_lo] - hi_base
    return KL, KH, slots_lo, slots_hi


def _pack(slots):
    """[NCORES, n_slots] -> [NCORES, 128, n_slots//16] int16 idx layout."""
    ncols = slots.shape[1] // 16
    if ncols == 0:
        return np.zeros((NCORES, 128, 0), np.int16)
    a = slots.reshape(NCORES, ncols, 16).transpose(0, 2, 1)
    a = a.astype(np.uint16).view(np.int16)
    return np.tile(a, (1, 8, 1))


def _preprocess(x, edge_index, n_real):
    n_tiles = -(-(n_real + 1) // P)
    n_tiles = -(-n_tiles // NCORES) * NCORES
    npad = n_tiles * P
    tpc = n_tiles // NCORES
    npc = tpc * P
    table_rows = 128 + npad + 128  # dummy tile, nodes, dummy tile
    assert table_rows <= 2 * WIN, "two int16 windows must cover the table"

    src0 = np.asarray(edge_index[0]).astype(np.int64)
    dst0 = np.asarray(edge_index[1]).astype(np.int64)

    deg = np.bincount(dst0, minlength=npad).astype(np.int64)
    deg[:n_real] += 1
    order = np.argsort(deg, kind="stable")
    pos = np.empty(npad, dtype=np.int64)
    pos[order] = np.arange(npad)
    tile_of = pos // P
    lane_of = pos % P
    r_of = tile_of // NCORES
    c_of = tile_of % NCORES
    new_id = c_of * npc + r_of * P + lane_of  # old -> global new id

    all_src = np.concatenate([new_id[src0], new_id[:n_real]])
    all_dst = np.concatenate([new_id[dst0], new_id[:n_real]])

    # layer-1 source rows: rotated per core (own block first), +128 offset
    blk = all_src // npc
    within = all_src % npc
    rot_rows = np.empty((len(all_src), NCORES), dtype=np.int64)
    for c in range(NCORES):
        rot_rows[:, c] = 128 + ((blk - c) % NCORES) * npc + within
    KL1, KH1, sl1, sh1 = _build_grids(rot_rows, all_dst, npc, tpc, table_rows)
    # layer-2 source rows: chunked-AllGather layout. The AG runs in NCH
    # chunks over round-blocks; chunk k of every core lands rank-major at
    # base_k. row(c, r, l) = 128 + base_k + c*rows_k + (r - r0_k)*128 + l.
    NCH = min(4, tpc)
    bounds = np.unique(np.linspace(0, tpc, NCH + 1).astype(int))
    NCH = len(bounds) - 1
    src_c = all_src // npc
    src_r = (all_src % npc) // P
    src_l = all_src % P
    chunk_of = np.searchsorted(bounds, src_r, side="right") - 1
    rows_k = (bounds[1:] - bounds[:-1]) * P
    base_k = np.concatenate([[0], np.cumsum(rows_k * NCORES)])
    l2_rows = (128 + base_k[chunk_of] + src_c * rows_k[chunk_of]
               + (src_r - bounds[chunk_of]) * P + src_l)
    KL2, KH2, sl2, sh2 = _build_grids(l2_rows, all_dst, npc, tpc, table_rows)

    idx1 = np.concatenate([_pack(sl1), _pack(sh1)], axis=2)
    idx2 = np.concatenate([_pack(sl2), _pack(sh2)], axis=2)

    return dict(
        npad=npad, npc=npc, tpc=tpc, table_rows=table_rows,
        ag_bounds=bounds,
        KL1=KL1, KH1=KH1, KL2=KL2, KH2=KH2,
        idx1=np.ascontiguousarray(idx1), idx2=np.ascontiguousarray(idx2),
        new_id=new_id, n_real=n_real,
    )


def _yfold(a_src):
    """Per-head transform T = D @ Q_house (y = T h, y0 = a_src . h) and its
    inverse R = T^{-1} as 128x128 block-diagonal f32 matrices."""
    a = np.asarray(a_src, np.float32)
    T = np.zeros((HC, HC), dtype=np.float64)
    R = np.zeros((HC, HC), dtype=np.float64)
    for h in range(H):
        ah = a[h].astype(np.float64)
        na = np.linalg.norm(ah)
        ahat = ah / na
        v = ahat - np.eye(C)[0]
        if np.linalg.norm(v) < 1e-12:
            Q = np.eye(C)
        else:
            v = v / np.linalg.norm(v)
            Q = np.eye(C) - 2.0 * np.outer(v, v)
        # Q is symmetric orthogonal with Q[0,:] = ahat
        Qo = Q.copy()
        Qo[0, :] = ahat  # guard sign: householder gives exactly this row
        D = np.eye(C)
        D[0, 0] = na
        Th = D @ Qo
        Rh = Qo.T @ np.diag([1.0 / na] + [1.0] * (C - 1))
        T[h * C:(h + 1) * C, h * C:(h + 1) * C] = Th
        R[h * C:(h + 1) * C, h * C:(h + 1) * C] = Rh
    return T.astype(np.float32), R.astype(np.float32)


def _wfull(W, a_src, a_dst):
    """[Wf @ T.T | Wf @ Wad] (132 cols, f16) plus rotate-back R (f16)."""
    W = np.asarray(W, dtype=np.float32)
    fin = W.shape[0]
    Wf = W.reshape(fin, HC)
    T, R = _yfold(a_src)
    Wad = np.zeros((HC, H), dtype=np.float32)
    for h in range(H):
        Wad[h * C:(h + 1) * C, h] = np.asarray(a_dst, np.float32)[h]
    out = np.concatenate([Wf @ T.T, Wf @ Wad], axis=1)  # [fin, 132]
    return (np.ascontiguousarray(out.astype(np.float16)),
            np.ascontiguousarray(R.astype(np.float16)))


def _dummy_row():
    row = np.zeros(128, dtype=np.float16)
    for h in range(H):
        row[h * C] = -60000.0
    return row.view(np.uint16)[None, :]


# ---------------------------------------------------------------------------
# device program
# ---------------------------------------------------------------------------

def _build(st, b1_zero, b2_zero):
    npc, tpc = st["npc"], st["tpc"]
    npad = st["npad"]
    table_rows = st["table_rows"]
    hi_base = max(0, table_rows - WIN)
    n_tiles = npad // P
    KLs = {1: st["KL1"], 2: st["KL2"]}
    KHs = {1: st["KH1"], 2: st["KH2"]}
    ncols = {1: st["idx1"].shape[2], 2: st["idx2"].shape[2]}
    locol = {
        1: 8 * int(st["KL1"].sum()),
        2: 8 * int(st["KL2"].sum()),
    }
    Kmax = max(
        int((st["KL1"] + st["KH1"]).max()), int((st["KL2"] + st["KH2"]).max())
    )
    idxcols = max(ncols[1], ncols[2], 16)

    nc = bacc.Bacc(None, target_bir_lowering=False,
                   dynamic_dma_scratch_size=RING)

    xT_in = nc.dram_tensor("xT", [F_IN, npad], f16, kind="ExternalInput")
    rot1_in = nc.dram_tensor("rot1", [HC, HC], f16, kind="ExternalInput")
    rot2_in = nc.dram_tensor("rot2", [HC, HC], f16, kind="ExternalInput")
    idx1_in = nc.dram_tensor("idx1", [128, idxcols], i16, kind="ExternalInput")
    idx2_in = nc.dram_tensor("idx2", [128, idxcols], i16, kind="ExternalInput")
    wfull1_in = nc.dram_tensor("wfull1", [F_IN, 132], f16, kind="ExternalInput")
    wfull2_in = nc.dram_tensor("wfull2", [HC, 132], f16, kind="ExternalInput")
    wc_in = nc.dram_tensor("wc", [HC, NCLS], f16, kind="ExternalInput")
    b1_in = nc.dram_tensor("b1", [1, HC], f32, kind="ExternalInput")
    b2_in = nc.dram_tensor("b2", [1, HC], f32, kind="ExternalInput")
    bc_in = nc.dram_tensor("bc", [1, NCLS], f32, kind="ExternalInput")
    dummy_in = nc.dram_tensor("dummyrow", [1, 128], u16, kind="ExternalInput")
    ident_in = nc.dram_tensor("ident16", [P, P], f16, kind="ExternalInput")

    logits_out = nc.dram_tensor("logits", [npc, NCLS], f32, kind="ExternalOutput")

    t_full1 = nc.dram_tensor("t_full1", [table_rows, 128], u16)
    agb = st["ag_bounds"]
    NCH = len(agb) - 1
    t2loc = [
        nc.dram_tensor(f"t2loc{k}", [(int(agb[k + 1]) - int(agb[k])) * P, 128],
                       u16)
        for k in range(NCH)
    ]
    t_full2 = nc.dram_tensor("t_full2", [table_rows, 128], u16, addr_space="Shared")

    rg = [list(range(NCORES))]

    with tile.TileContext(nc) as tc:
        with (
            tc.tile_pool(name="const", bufs=1) as constp,
            tc.tile_pool(name="xt", bufs=3) as xtp,
            tc.tile_pool(name="tstage", bufs=3) as tsp,
            tc.tile_pool(name="idx", bufs=2) as idxp,
            tc.tile_pool(name="ald", bufs=1) as aldp,
            tc.tile_pool(name="g", bufs=3) as gp,
            tc.tile_pool(name="m", bufs=2) as mp,
            tc.tile_pool(name="w32", bufs=3) as wp,
            tc.tile_pool(name="norm", bufs=4) as normp,
            tc.tile_pool(name="small", bufs=6) as smallp,
            tc.tile_pool(name="hkeep", bufs=1) as hkp,
            tc.tile_pool(name="cls", bufs=1) as clsp,
            tc.tile_pool(name="psd", bufs=2, space="PSUM") as psd,
            tc.tile_pool(name="pse", bufs=2, space="PSUM") as pse,
            tc.tile_pool(name="pst", bufs=1, space="PSUM") as pstp,
            tc.tile_pool(name="psc", bufs=1, space="PSUM") as pscp,
        ):
            ACT = mybir.ActivationFunctionType
            ident = constp.tile([P, P], f16)
            nc.sync.dma_start(out=ident[:, :], in_=ident_in[:, :])
            wf1 = constp.tile([F_IN, 132], f16)
            nc.sync.dma_start(out=wf1[:, :], in_=wfull1_in[:, :])
            wf2 = constp.tile([HC, 132], f16)
            nc.sync.dma_start(out=wf2[:, :], in_=wfull2_in[:, :])
            rot1 = constp.tile([HC, HC], f16, name="rot1", tag="rot1")
            nc.sync.dma_start(out=rot1[:, :], in_=rot1_in[:, :])
            rot2 = constp.tile([HC, HC], f16, name="rot2", tag="rot2")
            nc.sync.dma_start(out=rot2[:, :], in_=rot2_in[:, :])
            wc_sb = constp.tile([HC, NCLS], f16)
            nc.sync.dma_start(out=wc_sb[:, :], in_=wc_in[:, :])
            dummy_sb = constp.tile([1, 128], u16)
            nc.sync.dma_start(out=dummy_sb[:, :], in_=dummy_in[:, :])
            b_sb = {}
            for name, hnd, w in (("b1", b1_in, HC), ("b2", b2_in, HC),
                                 ("bc", bc_in, NCLS)):
                t = constp.tile([P, w], f32, name=f"bsb_{name}", tag=f"bsb_{name}")
                nc.sync.dma_start(out=t[:, :], in_=hnd[0:1, :].to_broadcast((P, w)))
                b_sb[name] = t
            bf16_1 = constp.tile([P, HC], f16, name="b1f16", tag="b1f16")
            nc.vector.tensor_copy(out=bf16_1[:, :], in_=b_sb["b1"][:, :])
            bf16_2 = constp.tile([P, HC], f16, name="b2f16", tag="b2f16")
            nc.vector.tensor_copy(out=bf16_2[:, :], in_=b_sb["b2"][:, :])

            # dummy rows of both tables
            for tf in (t_full1, t_full2):
                nc.sync.dma_start(out=tf[0:1, :], in_=dummy_sb[:, :])
                nc.sync.dma_start(
                    out=tf[table_rows - 1:table_rows, :], in_=dummy_sb[:, :]
                )

            # ---- replicated dense phase for layer 1 ----
            ald1 = aldp.tile([P, tpc, 4], f32, name="ald1", tag="ald1")
            ald2 = aldp.tile([P, tpc, 4], f32, name="ald2", tag="ald2")
            BT = 3   # tiles per PSUM batch
            GB = 12  # tiles per DMA group
            for gg in range(0, n_tiles, GB):
                gcnt = min(GB, n_tiles - gg)
                xt = xtp.tile([P, gcnt * P], f16, tag="xt")
                nc.sync.dma_start(
                    out=xt[:, :], in_=xT_in[:, gg * P:(gg + gcnt) * P]
                )
                ts_ = tsp.tile([P, gcnt, 128], u16, tag="ts")
                for g0 in range(gg, gg + gcnt, BT):
                    gn = min(BT, gg + gcnt - g0)
                    o = g0 - gg
                    ps = psd.tile([P, gn, 132], f32, tag="psd")
                    for j in range(gn):
                        nc.tensor.matmul(
                            out=ps[:, j, :],
                            lhsT=xt[:, (o + j) * P:(o + j + 1) * P],
                            rhs=wf1[:, :],
                            start=True, stop=True,
                        )
                    nc.scalar.activation(
                        out=ts_[:, o:o + gn, 0:128].bitcast(f16),
                        in_=ps[:, :, 0:128],
                        func=ACT.Copy,
                    )
                    if g0 < tpc:  # own tiles: extract a_dst . h
                        jn = min(gn, tpc - g0)
                        nc.vector.tensor_copy(
                            out=ald1[:, g0:g0 + jn, :], in_=ps[:, 0:jn, 128:132]
                        )
                nc.sync.dma_start(
                    out=t_full1[128 + gg * P:128 + (gg + gcnt) * P, :].rearrange(
                        "(g p) w -> p g w", p=P),
                    in_=ts_[:, :, :],
                )

            # ald for layer 2 is produced during layer-1 normalization.
            h1own = hkp.tile([P, tpc, HC], f16, name="h1own", tag="h1own")

            zsAll = clsp.tile([P, tpc, NCLS], f32, name="zsAll", tag="zsAll")
            smAll = clsp.tile([P, tpc], f32, name="smAll", tag="smAll")
            lnAll = clsp.tile([P, tpc], f32, name="lnAll", tag="lnAll")

            for layer in (1, 2):
                KL, KH = KLs[layer], KHs[layer]
                cumKL = np.concatenate([[0], np.cumsum(KL)])
                cumKH = np.concatenate([[0], np.cumsum(KH)])
                idx_sb = idxp.tile([128, idxcols], i16, tag="idx")
                nc.sync.dma_start(
                    out=idx_sb[:, :],
                    in_=(idx1_in if layer == 1 else idx2_in)[:, :],
                )
                t_full = t_full1 if layer == 1 else t_full2
                ald = ald1 if layer == 1 else ald2
                bias = bf16_1 if layer == 1 else bf16_2
                bzero = b1_zero if layer == 1 else b2_zero

                for r in range(tpc):
                    kl, kh = int(KL[r]), int(KH[r])
                    K = kl + kh
                    g = gp.tile([P, K, 128], u16, tag="g")
                    # lo gather
                    off = 0
                    col = 8 * int(cumKL[r])
                    while off < kl:
                        jn = min(MAXCH, kl - off)
                        nc.gpsimd.dma_gather(
                            out_ap=g[:, off:off + jn, :],
                            in_ap=t_full[0:min(WIN, table_rows), :],
                            idxs_ap=idx_sb[:, col:col + jn * 8],
                            num_idxs=jn * P,
                            num_idxs_reg=jn * P,
                            elem_size=128,
                        )
                        off += jn
                        col += jn * 8
                    # hi gather
                    off = 0
                    col = locol[layer] + 8 * int(cumKH[r])
                    while off < kh:
                        jn = min(MAXCH, kh - off)
                        nc.gpsimd.dma_gather(
                            out_ap=g[:, kl + off:kl + off + jn, :],
                            in_ap=t_full[hi_base:table_rows, :],
                            idxs_ap=idx_sb[:, col:col + jn * 8],
                            num_idxs=jn * P,
                            num_idxs_reg=jn * P,
                            elem_size=128,
                        )
                        off += jn
                        col += jn * 8

                    # w = exp(prelu(als_src + ald_dst))
                    w32 = wp.tile([P, K, 4], f32, tag="w32")
                    nc.vector.tensor_tensor(
                        out=w32[:, :, :],
                        in0=g[:, :, :].bitcast(f16).rearrange(
                            "p k (h c) -> p k h c", h=H)[:, :, :, 0],
                        in1=ald[:, r:r + 1, :].to_broadcast((P, K, 4)),
                        op=mybir.AluOpType.add,
                    )
                    zt = wp.tile([P, K, 4], f32, tag="zt")
                    if NO_PRELU:
                        nc.vector.tensor_scalar_mul(
                            out=zt[:, :, :], in0=w32[:, :, :], scalar1=NEG,
                        )
                        nc.vector.tensor_tensor(
                            out=zt[:, :, :], in0=w32[:, :, :], in1=zt[:, :, :],
                            op=mybir.AluOpType.max,
                        )
                    else:
                        nc.scalar.activation(
                            out=zt[:, :, :], in_=w32[:, :, :],
                            func=ACT.Prelu, alpha=NEG,
                        )
                    m = mp.tile([P, K, 132], f16, tag="m")
                    nc.scalar.activation(
                        out=m[:, :, 128:132], in_=zt[:, :, :], func=ACT.Exp,
                    )
                    a2 = wp.tile([P, K, 4, 2], f16, tag="a2")
                    nc.vector.tensor_copy(
                        out=a2[:, :, :, :],
                        in_=m[:, :, 128:132][:, :, :, None].to_broadcast(
                            (P, K, 4, 2)),
                    )
                    nc.vector.tensor_tensor(
                        out=m[:, :, 0:128].rearrange(
                            "p k (h c d) -> p k h c d", h=H, d=2),
                        in0=g[:, :, :].bitcast(f16).rearrange(
                            "p k (h c d) -> p k h c d", h=H, d=2),
                        in1=a2[:, :, :, None, :].to_broadcast((P, K, 4, 16, 2)),
                        op=mybir.AluOpType.mult,
                    )
                    # accumulate the round in PSUM via identity matmuls,
                    # two chunks per matmul (halves PE instruction count)
                    ps = pse.tile([P, 2, 132], f32, tag="pse")
                    if NO_PAIR:
                        nc.vector.memset(ps[:, 1, :], 0.0)
                        for j in range(K):
                            nc.tensor.matmul(
                                out=ps[:, 0, :], lhsT=ident[:, :], rhs=m[:, j, :],
                                start=(j == 0), stop=(j == K - 1),
                            )
                    else:
                        npairs = K // 2
                        for j in range(npairs):
                            nc.tensor.matmul(
                                out=ps[:, :, :], lhsT=ident[:, :],
                                rhs=m[:, 2 * j:2 * j + 2, :],
                                start=(j == 0),
                                stop=(j == npairs - 1 and K % 2 == 0),
                            )
                        if K % 2:
                            nc.tensor.matmul(
                                out=ps[:, 0, :], lhsT=ident[:, :],
                                rhs=m[:, K - 1, :],
                                start=False, stop=True,
                            )
                    acc0 = normp.tile([P, 132], f16, tag="acc0")
                    nc.scalar.activation(
                        out=acc0[:, :], in_=ps[:, 0, :], func=ACT.Copy,
                    )
                    accf = normp.tile([P, 132], f16, tag="accf")
                    nc.vector.tensor_tensor(
                        out=accf[:, :], in0=acc0[:, :], in1=ps[:, 1, :],
                        op=mybir.AluOpType.add,
                    )
                    # normalize + bias + leaky
                    den = smallp.tile([P, 4], f16, tag="den")
                    nc.vector.tensor_scalar_add(
                        out=den[:, :], in0=accf[:, 128:132], scalar1=1e-4,
                    )
                    rc2 = smallp.tile([P, 4, 2], f16, tag="rc2")
                    with nc.allow_low_precision(reason="f16 softmax denom"):
                        nc.vector.reciprocal(out=rc2[:, :, 0], in_=den[:, :])
                    nc.vector.tensor_copy(
                        out=rc2[:, :, 1], in_=rc2[:, :, 0],
                    )
                    xn = normp.tile([P, HC], f16, tag="xn")
                    nc.vector.tensor_tensor(
                        out=xn[:, :].rearrange("p (h c d) -> p h c d", h=H, d=2),
                        in0=accf[:, 0:128].rearrange(
                            "p (h c d) -> p h c d", h=H, d=2),
                        in1=rc2[:, :, None, :].to_broadcast((P, 4, 16, 2)),
                        op=mybir.AluOpType.mult,
                    )
                    ptry = pstp.tile([P, P], f16, tag="ptry")
                    nc.tensor.transpose(
                        out=ptry[:, :], in_=xn[:, :], identity=ident[:, :]
                    )
                    yT = normp.tile([P, P], f16, tag="yT")
                    nc.scalar.activation(
                        out=yT[:, :], in_=ptry[:, :], func=ACT.Copy,
                    )
                    prot = pstp.tile([P, HC], f32, tag="prot")
                    nc.tensor.matmul(
                        out=prot[:, :], lhsT=yT[:, :],
                        rhs=(rot1 if layer == 1 else rot2)[:, :],
                        start=True, stop=True,
                    )
                    xh = normp.tile([P, HC], f16, tag="xh")
                    nc.scalar.activation(
                        out=xh[:, :], in_=prot[:, :], func=ACT.Copy,
                    )
                    if not bzero:
                        nc.vector.tensor_tensor(
                            out=xh[:, :], in0=xh[:, :], in1=bias[:, :],
                            op=mybir.AluOpType.add,
                        )
                    hnext = normp.tile([P, HC], f16, tag="hnext")
                    if NO_PRELU:
                        nc.vector.tensor_scalar_mul(
                            out=hnext[:, :], in0=xh[:, :], scalar1=NEG,
                        )
                        nc.vector.tensor_tensor(
                            out=hnext[:, :], in0=xh[:, :], in1=hnext[:, :],
                            op=mybir.AluOpType.max,
                        )
                    else:
                        nc.scalar.activation(
                            out=hnext[:, :], in_=xh[:, :], func=ACT.Prelu,
                            alpha=NEG,
                        )

                    ptr = pstp.tile([P, P], f16, tag="ptr")
                    nc.tensor.transpose(
                        out=ptr[:, :], in_=hnext[:, :], identity=ident[:, :]
                    )
                    hT = normp.tile([P, P], f16, tag="hT")
                    nc.scalar.activation(
                        out=hT[:, :], in_=ptr[:, :], func=ACT.Copy,
                    )
                    if layer == 1:
                        nc.vector.tensor_copy(
                            out=h1own[:, r, :], in_=hnext[:, :]
                        )
                        ps2 = psd.tile([P, 1, 132], f32, tag="psd")
                        nc.tensor.matmul(
                            out=ps2[:, 0, :], lhsT=hT[:, :], rhs=wf2[:, :],
                            start=True, stop=True,
                        )
                        t2s = tsp.tile([P, 1, 128], u16, tag="t2s")
                        nc.scalar.activation(
                            out=t2s[:, 0, :].bitcast(f16),
                            in_=ps2[:, 0, 0:128], func=ACT.Copy,
                        )
                        nc.vector.tensor_copy(
                            out=ald2[:, r, :], in_=ps2[:, 0, 128:132]
                        )
                        kch = int(np.searchsorted(agb, r, side="right")) - 1
                        rr = r - int(agb[kch])
                        nc.sync.dma_start(
                            out=t2loc[kch][rr * P:(rr + 1) * P, :],
                            in_=t2s[:, 0, :],
                        )
                        if r == int(agb[kch + 1]) - 1:
                            nrows = (int(agb[kch + 1]) - int(agb[kch])) * P
                            base = 128 + sum(
                                (int(agb[j + 1]) - int(agb[j])) * P * NCORES
                                for j in range(kch))
                            nc.gpsimd.collective_compute(
                                "AllGather",
                                mybir.AluOpType.bypass,
                                ins=[t2loc[kch][:, :]],
                                outs=[t_full2[base:base + nrows * NCORES, :]],
                                replica_groups=rg,
                            )
                    else:
                        pc = pscp.tile([P, NCLS], f32, tag="pc")
                        nc.tensor.matmul(
                            out=pc[:, :], lhsT=hT[:, :], rhs=wc_sb[:, :],
                            start=True, stop=True,
                        )
                        lg = normp.tile([P, NCLS], f32, tag="lg")
                        nc.scalar.activation(
                            out=lg[:, :], in_=pc[:, :], func=ACT.Copy,
                        )
                        nc.vector.tensor_tensor(
                            out=lg[:, :], in0=lg[:, :], in1=b_sb["bc"][:, :],
                            op=mybir.AluOpType.add,
                        )
                        mx = smallp.tile([P, 1], f32, tag="mx")
                        nc.vector.reduce_max(
                            out=mx[:, :], in_=lg[:, :], axis=mybir.AxisListType.X
                        )
                        nc.vector.tensor_scalar(
                            out=zsAll[:, r, :], in0=lg[:, :], scalar1=mx[:, :],
                            scalar2=None, op0=mybir.AluOpType.subtract,
                        )
                        es = normp.tile([P, NCLS], f32, tag="lg2")
                        nc.scalar.activation(
                            out=es[:, :], in_=zsAll[:, r, :], func=ACT.Exp,
                        )
                        nc.vector.reduce_sum(
                            out=smAll[:, r:r + 1], in_=es[:, :],
                            axis=mybir.AxisListType.X,
                        )


            # ---- batched log-softmax tail ----
            nc.scalar.activation(
                out=lnAll[:, :], in_=smAll[:, :], func=ACT.Ln,
            )
            otAll = clsp.tile([P, tpc, NCLS], f32, name="otAll", tag="otAll")
            for r in range(tpc):
                nc.vector.tensor_scalar(
                    out=otAll[:, r, :], in0=zsAll[:, r, :],
                    scalar1=lnAll[:, r:r + 1], scalar2=None,
                    op0=mybir.AluOpType.subtract,
                )
            nc.sync.dma_start(
                out=logits_out[:, :].rearrange("(r l) c -> l r c", l=P),
                in_=otAll[:, :, :],
            )

    nc.finalize()
    return nc


# ---------------------------------------------------------------------------
# entry point
# ---------------------------------------------------------------------------

_CACHE = {}


def kernel(x, edge_index, W1, a1_src, a1_dst, b1, W2, a2_src, a2_dst, b2, Wc, bc):
    global LAST_EXEC_NS
    import os

    x = np.asarray(x, dtype=np.float32)
    n_real = x.shape[0]
    b1 = np.asarray(b1, dtype=np.float32)
    b2 = np.asarray(b2, dtype=np.float32)
    b1_zero = bool(np.all(b1 == 0))
    b2_zero = bool(np.all(b2 == 0))
    ekey = hash(np.asarray(edge_index).tobytes()) ^ hash((n_real, b1_zero, b2_zero))
    if ekey in _CACHE:
        nc, st = _CACHE[ekey]
    else:
        st = _preprocess(x, edge_index, n_real)
        nc = _build(st, b1_zero, b2_zero)
        _CACHE[ekey] = (nc, st)

    npad, npc, tpc = st["npad"], st["npc"], st["tpc"]
    new_id = st["new_id"]

    # per-core rotated xT (full table, f16)
    x_pad = np.zeros((npad, F_IN), dtype=np.float32)
    x_pad[new_id[:n_real]] = x
    x_blocks = x_pad.reshape(NCORES, npc, F_IN)

    wfull1, rot1 = _wfull(W1, a1_src, a1_dst)
    wfull2, rot2 = _wfull(W2, a2_src, a2_dst)
    wc = np.ascontiguousarray(np.asarray(Wc, dtype=np.float16))
    b1r = b1[None, :]
    b2r = b2[None, :]
    bcr = np.asarray(bc, dtype=np.float32)[None, :]
    dummy = _dummy_row()
    ident = np.eye(P, dtype=np.float16)

    idxcols = max(st["idx1"].shape[2], st["idx2"].shape[2], 16)

    def pad_idx(a):
        if a.shape[2] < idxcols:
            a = np.concatenate(
                [a, np.zeros((NCORES, 128, idxcols - a.shape[2]), np.int16)],
                axis=2)
        return a

    idx1 = pad_idx(st["idx1"])
    idx2 = pad_idx(st["idx2"])

    in_maps = []
    for c in range(NCORES):
        rot = np.roll(np.arange(NCORES), -c)  # own block first
        xT = np.ascontiguousarray(
            x_blocks[rot].reshape(npad, F_IN).T.astype(np.float16)
        )
        in_maps.append({
            "xT": xT,
            "idx1": np.ascontiguousarray(idx1[c]),
            "idx2": np.ascontiguousarray(idx2[c]),
            "wfull1": wfull1, "wfull2": wfull2, "wc": wc,
            "rot1": rot1, "rot2": rot2,
            "b1": b1r, "b2": b2r, "bc": bcr,
            "dummyrow": dummy, "ident16": ident,
        })

    os.environ.setdefault("BASS_NEVER_TRACE", "1")
    res = run_bass_kernel_spmd(nc, in_maps, core_ids=list(range(NCORES)))
    LAST_EXEC_NS = res.exec_time_ns

    logits_pad = np.concatenate(
        [res.results[c]["logits"] for c in range(NCORES)], axis=0
    )
    return logits_pad[new_id[:n_real]].astype(np.float32)
